# revision 8
# baseline (speedup 1.0000x reference)
"""MoE mesh-deformer decoder on 8 trn2 NeuronCores.

Parallelization: output-feature slicing (tensor parallelism). Each of the 8
cores holds all 16 experts' weights for a 128-wide slice of the 1024 output
features (31.5 MB/core instead of 252 MB replicated), computes its slice of
every layer, and the full activation is reassembled with an AllGather between
layers. The final layer needs no collective: each core emits its slice and the
host concatenates.

Compute orientation is transposed ([features, batch]) so every matmul has a
256-wide moving operand, which lets fp32 data run through the PE array as
float32r (FP22) at full bf16 rate. The per-sample expert coefficients are
applied on eviction: coeff rows are broadcast across partitions with one K=16
matmul per expert (indicator x coeffT), then a DVE multiply-accumulate.
"""

import numpy as np

B = 256          # batch
LZ = 256         # z width
CW = 1024        # c width
IN = LZ + CW     # 1280, contraction width of every expert layer
HID = 1024       # expert layer output width
E = 16           # experts
GH = 64          # gating hidden
NCORES = 8
SL = HID // NCORES   # 128 output features per core
KT = IN // 128       # 10 K-tiles

_CACHE = {}


def _build():
    from concourse import bacc, tile, mybir
    from concourse.masks import make_identity

    F32 = mybir.dt.float32
    F32R = mybir.dt.float32r
    ADD = mybir.AluOpType.add
    MULT = mybir.AluOpType.mult
    BYPASS = mybir.AluOpType.bypass
    EXP = mybir.ActivationFunctionType.Exp
    AXX = mybir.AxisListType.X
    RG = [list(range(NCORES))]

    nc = bacc.Bacc("TRN2", target_bir_lowering=False, debug=False,
                   num_devices=NCORES)

    xT_d = nc.dram_tensor("xT", [IN, B], F32, kind="ExternalInput").ap()
    wt_d = nc.dram_tensor("wt", [3, E, 128, KT * 128], F32,
                          kind="ExternalInput").ap()
    bt_d = nc.dram_tensor("bt", [E, 3 * SL], F32, kind="ExternalInput").ap()
    gw0_d = nc.dram_tensor("gw0t", [128, KT * GH], F32,
                           kind="ExternalInput").ap()
    gw1_d = nc.dram_tensor("gw1", [GH, GH], F32, kind="ExternalInput").ap()
    gw2_d = nc.dram_tensor("gw2", [GH, E], F32, kind="ExternalInput").ap()
    gb0_d = nc.dram_tensor("gb0", [GH, 1], F32, kind="ExternalInput").ap()
    gb1_d = nc.dram_tensor("gb1", [GH, 1], F32, kind="ExternalInput").ap()
    gb2_d = nc.dram_tensor("gb2", [E, 1], F32, kind="ExternalInput").ap()
    ind_d = nc.dram_tensor("ind", [E, E * 128], F32, kind="ExternalInput").ap()
    out_d = nc.dram_tensor("outT", [SL, B], F32, kind="ExternalOutput").ap()

    with tile.TileContext(nc) as tc:
        with tc.tile_pool(name="sb", bufs=1) as sb, \
             tc.tile_pool(name="wp", bufs=12) as wp, \
             tc.tile_pool(name="eps", bufs=6, space="PSUM") as epsp, \
             tc.tile_pool(name="msps", bufs=2, space="PSUM") as msps, \
             tc.tile_pool(name="dram", bufs=1, space="DRAM") as dramp:

            # Fire a tiny collective immediately: the first collective of a
            # NEFF execution pays the cross-core rank-sync barrier; doing it
            # up front overlaps that wait with the layer-0 weight DMA.
            ccd_in = dramp.tile([1, 8], F32, tag="ccdi")
            ccd_out = dramp.tile([8, 8], F32, addr_space="Shared", tag="ccdo")
            nc.gpsimd.collective_compute(
                "AllGather", BYPASS, replica_groups=RG,
                ins=[ccd_in.opt()], outs=[ccd_out.opt()])

            identity = sb.tile([128, 128], F32)
            make_identity(nc, identity)
            # ind[:, e*128:(e+1)*128] is the K=16 indicator of expert e: the
            # matmul ind_e.T @ coeffT broadcasts coeff row e to 128 partitions.
            ind = sb.tile([E, E * 128], F32)
            nc.sync.dma_start(ind[:], ind_d)

            # PE warm-keeper: dummy fp32 matmuls (no consumers) emitted at
            # kernel start and across the AllGather waits so the HAM clock
            # gate stays at 8/8. fp32 runs 4 cycles/row, so each N=512 dummy
            # buys ~0.9us (warm) of PE busy time.
            warm = sb.tile([128, 512], F32)
            nc.vector.memset(warm[:], 0.0)

            def pe_warm(n):
                wm = msps.tile([128, 512], F32, tag="misc", name="wm")
                for _ in range(n):
                    nc.tensor.matmul(wm[:], warm[:, :128], warm[:],
                                     start=True, stop=True)

            pe_warm(4)

            x0 = sb.tile([128, KT, B], F32R)
            nc.sync.dma_start(x0[:], xT_d.rearrange("(k p) n -> p k n",
                                                    p=128).bitcast(F32R))
            gw0 = sb.tile([128, KT, GH], F32R)
            nc.sync.dma_start(gw0[:], gw0_d.rearrange("p (k m) -> p k m",
                                                      k=KT).bitcast(F32R))
            gw1 = sb.tile([GH, GH], F32)
            nc.sync.dma_start(gw1[:], gw1_d)
            gw2 = sb.tile([GH, E], F32)
            nc.sync.dma_start(gw2[:], gw2_d)
            gb0 = sb.tile([GH, 1], F32)
            nc.sync.dma_start(gb0[:], gb0_d)
            gb1 = sb.tile([GH, 1], F32)
            nc.sync.dma_start(gb1[:], gb1_d)
            gb2 = sb.tile([E, 1], F32)
            nc.sync.dma_start(gb2[:], gb2_d)
            bt = sb.tile([E, 3 * SL], F32)
            nc.sync.dma_start(bt[:], bt_d)

            def elu(out_ap, u_ap, tagpfx):
                # elu(u) = max(u,0) + exp(min(u,0)) - 1
                r = sb.tile(list(u_ap.shape), F32, tag=tagpfx + "r", bufs=2,
                            name=tagpfx + "r")
                nc.vector.tensor_scalar_max(r[:], u_ap, 0.0)
                n_ = sb.tile(list(u_ap.shape), F32, tag=tagpfx + "n", bufs=2,
                             name=tagpfx + "n")
                nc.vector.tensor_scalar_min(n_[:], u_ap, 0.0)
                ex = sb.tile(list(u_ap.shape), F32, tag=tagpfx + "e", bufs=2,
                             name=tagpfx + "e")
                nc.scalar.activation(ex[:], n_[:], EXP)
                nc.vector.scalar_tensor_tensor(
                    out_ap, in0=ex[:], scalar=-1.0, in1=r[:],
                    op0=ADD, op1=ADD)

            # ---- gating network (transposed, replicated on every core) ----
            g0ps = msps.tile([GH, B], F32, tag="misc")
            for k in range(KT):
                nc.tensor.matmul(g0ps[:], gw0[:, k, :], x0[:, k, :],
                                 start=(k == 0), stop=(k == KT - 1))
            g0u = sb.tile([GH, B], F32, tag="gu", bufs=2)
            nc.vector.tensor_scalar_add(g0u[:], g0ps[:], gb0[:])
            g0 = sb.tile([GH, B], F32, tag="gact", bufs=2)
            elu(g0[:], g0u[:], "g0")

            g1ps = msps.tile([GH, B], F32, tag="misc")
            nc.tensor.matmul(g1ps[:], gw1[:], g0[:], start=True, stop=True)
            g1u = sb.tile([GH, B], F32, tag="gu", bufs=2)
            nc.vector.tensor_scalar_add(g1u[:], g1ps[:], gb1[:])
            g1 = sb.tile([GH, B], F32, tag="gact", bufs=2)
            elu(g1[:], g1u[:], "g1")

            lps = msps.tile([E, B], F32, tag="misc")
            nc.tensor.matmul(lps[:], gw2[:], g1[:], start=True, stop=True)
            lg = sb.tile([E, B], F32)
            nc.vector.tensor_scalar_add(lg[:], lps[:], gb2[:])

            # softmax over experts, done batch-major so the max/sum are
            # free-dim reductions exactly matching jax.nn.softmax numerics
            co = sb.tile([128, 2, E], F32)
            for m in range(2):
                trp = msps.tile([128, E], F32, tag="misc")
                nc.tensor.transpose(trp[:], lg[:, m * 128:(m + 1) * 128],
                                    identity[:E, :E])
                mx = sb.tile([128, 1], F32, tag="mx", bufs=2)
                nc.vector.reduce_max(mx[:], trp[:], axis=AXX)
                sm = sb.tile([128, E], F32, tag="sm", bufs=2)
                nc.vector.tensor_scalar(sm[:], trp[:], mx[:], None,
                                        op0=mybir.AluOpType.subtract)
                exs = sb.tile([128, E], F32, tag="exs", bufs=2)
                nc.scalar.activation(exs[:], sm[:], EXP)
                sume = sb.tile([128, 1], F32, tag="sume", bufs=2)
                nc.vector.reduce_sum(sume[:], exs[:], axis=AXX)
                rcp = sb.tile([128, 1], F32, tag="rcp", bufs=2)
                nc.vector.reciprocal(rcp[:], sume[:])
                nc.vector.tensor_scalar_mul(co[:, m, :], exs[:], rcp[:])

            # coeffT [E, B]: transpose back
            coTp = msps.tile([E, B], F32, tag="misc")
            for m in range(2):
                nc.tensor.transpose(coTp[:, m * 128:(m + 1) * 128],
                                    co[:, m, :], identity[:])
            coT = sb.tile([E, B], F32)
            nc.vector.tensor_copy(coT[:], coTp[:])

            # bcast[p, e, n] = coeff[n, e] for every partition p
            bc = sb.tile([128, E, B], F32)
            for e2 in range(E // 2):
                bcp = epsp.tile([128, 2 * B], F32, tag="eps", name="bcp")
                for h in range(2):
                    e = 2 * e2 + h
                    nc.tensor.matmul(bcp[:, h * B:(h + 1) * B],
                                     ind[:, e * 128:(e + 1) * 128], coT[:],
                                     start=True, stop=True)
                nc.vector.tensor_copy(bc[:, 2 * e2, :], bcp[:, :B])
                nc.vector.tensor_copy(bc[:, 2 * e2 + 1, :], bcp[:, B:])

            # ---- expert layers ----
            gat = [None, None]
            cc_in = [None, None]
            cc_out = [None, None]
            for l in range(2):
                cc_in[l] = dramp.tile([SL, B], F32, tag=f"cci{l}",
                                      name=f"cci{l}")
                cc_out[l] = dramp.tile([NCORES * SL, B], F32,
                                       addr_space="Shared", tag=f"cco{l}",
                                       name=f"cco{l}")

            for l in range(3):
                if l > 0:
                    pe_warm(44)
                if l == 0:
                    rhs = [x0[:, k, :] for k in range(KT)]
                else:
                    rhs = [x0[:, 0, :], x0[:, 1, :]] + \
                          [gat[l - 1][:, k, :] for k in range(8)]

                biasps = epsp.tile([128, B], F32, tag="eps", name="biasps")
                nc.tensor.matmul(biasps[:], bt[:, l * SL:(l + 1) * SL],
                                 coT[:], start=True, stop=True)
                acc = sb.tile([128, B], F32, tag="acc", bufs=2, name="acc")
                nc.vector.tensor_copy(acc[:], biasps[:])

                for e in range(E):
                    w_sb = wp.tile([128, KT * 128], F32R, tag="w",
                                   name="w_sb")
                    dma_eng = nc.sync if e % 2 == 0 else nc.scalar
                    dma_eng.dma_start(w_sb[:], wt_d[l, e].bitcast(F32R))
                    ps = epsp.tile([128, B], F32, tag="eps", name="ps")
                    for k in range(KT):
                        nc.tensor.matmul(
                            ps[:], w_sb[:, k * 128:(k + 1) * 128], rhs[k],
                            start=(k == 0), stop=(k == KT - 1))
                    tmp = sb.tile([128, B], F32, tag="tmp", bufs=3, name="tmp")
                    nc.vector.tensor_tensor(out=tmp[:], in0=ps[:],
                                            in1=bc[:, e, :], op=MULT)
                    nc.vector.tensor_tensor(out=acc[:], in0=acc[:],
                                            in1=tmp[:], op=ADD)

                if l < 2:
                    act = sb.tile([128, B], F32, tag="act", bufs=2, name="act")
                    elu(act[:], acc[:], "L")
                    nc.sync.dma_start(cc_in[l][:], act[:])
                    nc.gpsimd.collective_compute(
                        "AllGather", BYPASS, replica_groups=RG,
                        ins=[cc_in[l].opt()], outs=[cc_out[l].opt()])
                    gat[l] = sb.tile([128, 8, B], F32R, tag=f"gat{l}",
                                     name=f"gat{l}")
                    cc_re = cc_out[l].rearrange("(k p) n -> p k n",
                                                p=128).bitcast(F32R)
                    for k in range(8):
                        eng = nc.sync if k % 2 == 0 else nc.scalar
                        eng.dma_start(gat[l][:, k, :], cc_re[:, k, :])
                else:
                    nc.sync.dma_start(out_d, acc[:])

    nc.compile()
    return nc


def _get_program():
    if "nc" not in _CACHE:
        _CACHE["nc"] = _build()
    return _CACHE["nc"]


def _prep_inputs(z, c, w0, b0, w1, b1, w2, b2, gw0, gb0, gw1, gb1, gw2, gb2):
    f = lambda a: np.ascontiguousarray(np.asarray(a, dtype=np.float32))
    z, c = f(z), f(c)
    xT = np.ascontiguousarray(np.concatenate([z, c], axis=1).T)  # [1280, 256]
    gw0t = np.ascontiguousarray(
        f(gw0).reshape(KT, 128, GH).transpose(1, 0, 2).reshape(128, KT * GH))
    gw1 = f(gw1)
    gw2 = f(gw2)
    gb0c = f(gb0).reshape(GH, 1)
    gb1c = f(gb1).reshape(GH, 1)
    gb2c = f(gb2).reshape(E, 1)
    indm = np.ascontiguousarray(
        np.kron(np.eye(E, dtype=np.float32), np.ones((1, 128), np.float32)))
    ws = [f(w0), f(w1), f(w2)]
    bs = [f(b0), f(b1), f(b2)]

    in_maps = []
    for j in range(NCORES):
        lo, hi = j * SL, (j + 1) * SL
        wt = np.empty((3, E, 128, KT * 128), np.float32)
        bt = np.empty((E, 3 * SL), np.float32)
        for l in range(3):
            # [E, IN, SL] -> [E, KT, 128, SL] -> [E, 128, KT, SL]
            wt[l] = (ws[l][:, :, lo:hi]
                     .reshape(E, KT, 128, SL)
                     .transpose(0, 2, 1, 3)
                     .reshape(E, 128, KT * 128))
            bt[:, l * SL:(l + 1) * SL] = bs[l][:, lo:hi]
        in_maps.append({
            "xT": xT, "wt": np.ascontiguousarray(wt),
            "bt": np.ascontiguousarray(bt),
            "gw0t": gw0t, "gw1": gw1, "gw2": gw2,
            "gb0": gb0c, "gb1": gb1c, "gb2": gb2c, "ind": indm,
        })
    return in_maps


def run_on_device(inputs, trace=False, **kw):
    """Run the bass program; returns (full_output [256,1024], BassKernelResults)."""
    from concourse import bass_utils
    nc = _get_program()
    in_maps = _prep_inputs(**inputs)
    res = bass_utils.run_bass_kernel_spmd(
        nc, in_maps, core_ids=list(range(NCORES)), trace=trace, **kw)
    slices = [r["outT"] for r in res.results]          # each [128, 256]
    full = np.concatenate(slices, axis=0).T            # [256, 1024]
    return np.ascontiguousarray(full, dtype=np.float32), res


def kernel(**inputs):
    out, _ = run_on_device(inputs)
    return out
